# revision 1
# baseline (speedup 1.0000x reference)
"""Trainium2 Bass kernel for nn_MatchNet (MLP forward + 60-iter batched PDHG LP solve).

Data-parallel over 8 NeuronCores: batch 2048 -> 256 rows/core. MLP weights and
S are replicated. Each core runs the full unrolled PDHG solve on its shard.

Math (per core, batch rows b=256, n=512 structures, m=64 combos):
    Z = relu(relu(relu(X@W1+b1)@W2+b2)@W3+b3)          (computed in T layout)
    tau = sigma = 0.9/L,  alpha = tau*sigma            (L from host power iter)
    scaled duals p = tau*y1 [m,b]T, q = tau*y2, states e = x-Z, eb = xbar-Z:
      p+  = relu(p + alpha*(S@xbarT - BT))             xbar = Z + eb
      q+  = relu(q - alpha*(Z + eb))
      t1  = p+@S - q+                                  (PSUM)
      d   = e - t1 + tau
      n2  = sum_feat d^2 ; s = relu(1 - tau/max(sqrt(n2),1e-12))
      m_  = s*d ;  e+ = m_ ;  eb+ = 2*m_ - e
    out x = Z + e_final

Layouts: N-folded SBUF tiles [128, 1024]: col b*512+f = (batch 128*bt+p, feat f).
T-folded [128, 1024]: col c*256+j = (feat c*128+p, batch j).
"""

import numpy as np

N_STRUCTS = 512
N_COMBOS = 64
HID = 1024
N_ITERS = 60
N_CORES = 8
B_FULL = 2048
BC = B_FULL // N_CORES  # 256 batch rows per core
NB = BC // 128  # 2 batch sub-tiles
NF = N_STRUCTS // 128  # 4 feature chunks

_BUILD_CACHE = {}


def _power_L(S: np.ndarray) -> float:
    """Replicates reference.py's power iteration for ||K||_2 in float32."""
    S = S.astype(np.float32)
    n = S.shape[1]
    v = np.full((n,), 1.0 / np.sqrt(n), np.float32)
    for _ in range(30):
        v2 = (S.T @ (S @ v) + v).astype(np.float32)
        v = (v2 / np.float32(np.linalg.norm(v2))).astype(np.float32)
    L = np.sqrt(np.vdot(v, (S.T @ (S @ v) + v).astype(np.float32)))
    return float(L)


def _build_nc(tau: float, sigma: float):
    import contextlib

    import concourse.bacc as bacc
    import concourse.mybir as mybir
    import concourse.tile as tile

    f32 = mybir.dt.float32
    AF = mybir.ActivationFunctionType
    ALU = mybir.AluOpType
    alpha = tau * sigma

    nc = bacc.Bacc("TRN2", target_bir_lowering=False, debug=False)

    # ---- DRAM I/O (per-core shapes) ----
    d_XT = nc.dram_tensor("xt", [N_COMBOS, BC], f32, kind="ExternalInput")
    d_W1 = nc.dram_tensor("w1", [N_COMBOS, HID], f32, kind="ExternalInput")
    d_b1 = nc.dram_tensor("b1r", [128, 8], f32, kind="ExternalInput")
    d_W2 = nc.dram_tensor("w2", [HID, HID], f32, kind="ExternalInput")
    d_b2 = nc.dram_tensor("b2r", [128, 8], f32, kind="ExternalInput")
    d_W3 = nc.dram_tensor("w3", [HID, N_STRUCTS], f32, kind="ExternalInput")
    d_b3 = nc.dram_tensor("b3r", [128, 4], f32, kind="ExternalInput")
    d_S = nc.dram_tensor("s", [N_COMBOS, N_STRUCTS], f32, kind="ExternalInput")
    d_aST = nc.dram_tensor("ast", [128, 4 * N_COMBOS], f32, kind="ExternalInput")
    d_I = nc.dram_tensor("ident", [128, 128], f32, kind="ExternalInput")
    d_out = nc.dram_tensor("out", [BC, N_STRUCTS], f32, kind="ExternalOutput")

    FW = N_STRUCTS  # 512 per-b tile width

    with tile.TileContext(nc) as tc:
        stack = contextlib.ExitStack()
        with stack:
            cpool = stack.enter_context(tc.tile_pool(name="consts", bufs=1))

            def cload(dram, shape, tag):
                t = cpool.tile(shape, f32, tag=tag)
                nc.sync.dma_start(t[:], dram.ap())
                return t

            XT = cload(d_XT, [N_COMBOS, BC], "xt")
            W1 = cload(d_W1, [N_COMBOS, HID], "w1")
            b1r = cload(d_b1, [128, 8], "b1r")
            b2r = cload(d_b2, [128, 8], "b2r")
            b3r = cload(d_b3, [128, 4], "b3r")
            S_sb = cload(d_S, [N_COMBOS, N_STRUCTS], "s")
            aST = cload(d_aST, [128, 4 * N_COMBOS], "ast")
            I128 = cload(d_I, [128, 128], "ident")
            # ---- MLP forward, T layout ----
            zt = []  # Z^T tiles [128, BC] x4
            with (
                tc.tile_pool(name="mlp_sb", bufs=1) as mpool,
                tc.tile_pool(name="mlp_ps", bufs=4, space="PSUM") as mpsum,
            ):
                W2 = []
                for k in range(8):
                    t = mpool.tile([128, HID], f32, tag=f"w2_{k}", name=f"w2_{k}")
                    nc.sync.dma_start(t[:], d_W2.ap()[k * 128 : (k + 1) * 128, :])
                    W2.append(t)
                W3 = []
                for k in range(8):
                    t = mpool.tile([128, N_STRUCTS], f32, tag=f"w3_{k}", name=f"w3_{k}")
                    nc.sync.dma_start(t[:], d_W3.ap()[k * 128 : (k + 1) * 128, :])
                    W3.append(t)
                z1t = []
                for t in range(8):
                    ps = mpsum.tile([128, BC], f32, tag="mm")
                    nc.tensor.matmul(
                        ps[:], W1[:, t * 128 : (t + 1) * 128], XT[:], start=True, stop=True
                    )
                    sb = mpool.tile([128, BC], f32, tag=f"z1_{t}")
                    nc.scalar.activation(sb[:], ps[:], AF.Relu, bias=b1r[:, t : t + 1])
                    z1t.append(sb)
                z2t = []
                for t in range(8):
                    ps = mpsum.tile([128, BC], f32, tag="mm")
                    for k in range(8):
                        nc.tensor.matmul(
                            ps[:],
                            W2[k][:, t * 128 : (t + 1) * 128],
                            z1t[k][:],
                            start=(k == 0),
                            stop=(k == 7),
                        )
                    sb = mpool.tile([128, BC], f32, tag=f"z2_{t}")
                    nc.scalar.activation(sb[:], ps[:], AF.Relu, bias=b2r[:, t : t + 1])
                    z2t.append(sb)
                for c in range(NF):
                    ps = mpsum.tile([128, BC], f32, tag="mm")
                    for k in range(8):
                        nc.tensor.matmul(
                            ps[:],
                            W3[k][:, c * 128 : (c + 1) * 128],
                            z2t[k][:],
                            start=(k == 0),
                            stop=(k == 7),
                        )
                    sb = cpool.tile([128, BC], f32, tag=f"zt_{c}")
                    nc.scalar.activation(sb[:], ps[:], AF.Relu, bias=b3r[:, c : c + 1])
                    zt.append(sb)

            # ---- PDHG setup ----
            spool = stack.enter_context(tc.tile_pool(name="setup", bufs=1))
            with tc.tile_pool(name="pd_ps", bufs=1, space="PSUM") as ppool:
                # cSZB = alpha*S@Z^T - alpha*B^T   [64, BC]
                ps = ppool.tile([N_COMBOS, BC], f32, tag="py1")
                for c in range(NF):
                    nc.tensor.matmul(
                        ps[:], aST[:, c * 64 : (c + 1) * 64], zt[c][:],
                        start=(c == 0), stop=False,
                    )
                naI64 = spool.tile([N_COMBOS, N_COMBOS], f32, tag="nai64")
                nc.scalar.activation(naI64[:], I128[:64, :64], AF.Copy, scale=-alpha)
                nc.tensor.matmul(ps[:], naI64[:], XT[:], start=False, stop=True)
                cSZB = spool.tile([N_COMBOS, BC], f32, tag="cszb")
                nc.scalar.activation(cSZB[:], ps[:], AF.Copy)

                # Z per-b in N layout via PE transposes
                Z, naZ = [], []
                for b in range(NB):
                    psz = ppool.tile([128, FW], f32, tag=f"pz{b}")
                    for c in range(NF):
                        nc.tensor.transpose(
                            psz[:, c * 128 : (c + 1) * 128],
                            zt[c][:, b * 128 : (b + 1) * 128],
                            I128[:],
                        )
                    zb = spool.tile([128, FW], f32, tag=f"zn{b}")
                    nc.scalar.activation(zb[:], psz[:], AF.Copy)
                    Z.append(zb)
                    nb_ = spool.tile([128, FW], f32, tag=f"naz{b}")
                    nc.scalar.activation(nb_[:], zb[:], AF.Copy, scale=-alpha)
                    naZ.append(nb_)

            # ---- PDHG state pools ----
            em_pool = stack.enter_context(tc.tile_pool(name="em", bufs=4))
            eb_pool = stack.enter_context(tc.tile_pool(name="eb", bufs=4))
            p_pool = stack.enter_context(tc.tile_pool(name="pp", bufs=4))
            q_pool = stack.enter_context(tc.tile_pool(name="qq", bufs=4))
            sc_pool = stack.enter_context(tc.tile_pool(name="scratch", bufs=4))
            ps_T = stack.enter_context(tc.tile_pool(name="ps_T", bufs=2, space="PSUM"))
            ps_y1 = stack.enter_context(tc.tile_pool(name="ps_y1", bufs=1, space="PSUM"))
            ps_t1 = stack.enter_context(tc.tile_pool(name="ps_t1", bufs=1, space="PSUM"))

            e, eb, pc, q = [], [], [], []
            for b in range(NB):
                t = em_pool.tile([128, FW], f32, tag=f"em{b}")
                nc.scalar.activation(t[:], Z[b][:], AF.Copy, scale=-1.0, bias=tau)
                e.append(t)
                t = eb_pool.tile([128, FW], f32, tag=f"eb{b}")
                nc.scalar.activation(t[:], Z[b][:], AF.Copy, scale=-1.0)
                eb.append(t)
                t = p_pool.tile([N_COMBOS, 128], f32, tag=f"pc{b}")
                nc.vector.tensor_copy(t[:], cSZB[:, b * 128 : (b + 1) * 128])
                pc.append(t)
                t = q_pool.tile([128, FW], f32, tag=f"q{b}")
                nc.gpsimd.memset(t[:], 0.0)
                q.append(t)


            import contextlib as _ctx

            def shift(b):
                # phase-shift chain b0 half an iteration earlier in priority
                return _ctx.nullcontext()

            for it in range(N_ITERS):
                psT, ebT, ps1, p_new, pc_new = [None] * NB, [None] * NB, [None] * NB, [None] * NB, [None] * NB
                w_, h_, q_new, ps3, u = [None] * NB, [None] * NB, [None] * NB, [None] * NB, [None] * NB
                d, n2, dsq, nmax, nrm = [None] * NB, [None] * NB, [None] * NB, [None] * NB, [None] * NB
                rr, rs, s, m_, eb_new = [None] * NB, [None] * NB, [None] * NB, [None] * NB, [None] * NB

                # q+ = relu((q - alpha*Z) - alpha*eb) ; w is off-critical-path
                for b in range(NB):
                    with shift(b):
                        w_[b] = sc_pool.tile([128, FW], f32, tag=f"w{b}", name=f"w{b}")
                        nc.gpsimd.tensor_add(w_[b][:], q[b][:], naZ[b][:])
                first_T = {}
                for b in range(NB):
                    with shift(b):
                        psT[b] = ps_T.tile([128, FW], f32, tag=f"pT{b}", name=f"pT_{b}")
                        for c in range(NF):
                            ti = nc.tensor.transpose(
                                psT[b][:, c * 128 : (c + 1) * 128],
                                eb[b][:, c * 128 : (c + 1) * 128],
                                I128[:],
                            )
                            if c == 0:
                                first_T[b] = ti
                for b in range(NB):
                    with shift(b):
                        ebT[b] = sc_pool.tile([128, FW], f32, tag=f"ebT{b}", name=f"ebT{b}")
                        nc.scalar.activation(ebT[b][:, 0:256], psT[b][:, 0:256], AF.Copy)
                        nc.scalar.activation(ebT[b][:, 256:512], psT[b][:, 256:512], AF.Copy)
                for b in range(NB):
                    with shift(b):
                        h_[b] = sc_pool.tile([128, FW], f32, tag=f"h{b}", name=f"h{b}")
                        nc.vector.scalar_tensor_tensor(
                            h_[b][:], eb[b][:], -alpha, w_[b][:], op0=ALU.mult, op1=ALU.add
                        )
                for b in range(NB):
                    with shift(b):
                        ps1[b] = ps_y1.tile([N_COMBOS, 128], f32, tag=f"py1{b}", name=f"py1_{b}")
                        nc.tensor.matmul(ps1[b][:], I128[:64, :64], pc[b][:], start=True, stop=False)
                        for c in range(NF):
                            nc.tensor.matmul(
                                ps1[b][:],
                                aST[:, c * 64 : (c + 1) * 64],
                                ebT[b][:, c * 128 : (c + 1) * 128],
                                start=False, stop=(c == NF - 1),
                            )
                for b in range(NB):
                    with shift(b):
                        q_new[b] = q_pool.tile([128, FW], f32, tag=f"q{b}", name=f"q{b}")
                        if b == 0:
                            nc.vector.tensor_scalar_max(q_new[b][:], h_[b][:], 0.0)
                        else:
                            nc.scalar.activation(q_new[b][:], h_[b][:], AF.Relu)
                for b in range(NB):
                    with shift(b):
                        p_new[b] = p_pool.tile([N_COMBOS, 128], f32, tag=f"p{b}", name=f"p{b}")
                        nc.scalar.activation(p_new[b][:], ps1[b][:], AF.Relu)
                        pc_new[b] = p_pool.tile([N_COMBOS, 128], f32, tag=f"pc{b}", name=f"pc{b}")
                        nc.gpsimd.tensor_add(
                            pc_new[b][:], p_new[b][:], cSZB[:, b * 128 : (b + 1) * 128]
                        )
                for b in range(NB):
                    with shift(b):
                        u[b] = sc_pool.tile([128, FW], f32, tag=f"u{b}", name=f"u{b}")
                        if b == 0:
                            nc.vector.tensor_add(u[b][:], e[b][:], q_new[b][:])
                        else:
                            nc.gpsimd.tensor_add(u[b][:], e[b][:], q_new[b][:])
                p1_inst = {}
                for b in range(NB):
                    with shift(b):
                        ps3[b] = ps_t1.tile([128, FW], f32, tag=f"pt1{b}", name=f"pt1_{b}")
                        p1_inst[b] = nc.tensor.matmul(
                            ps3[b][:], p_new[b][:], S_sb[:], start=True, stop=True
                        )


                for b in range(NB):
                    with shift(b):
                        d[b] = sc_pool.tile([128, FW], f32, tag=f"d{b}", name=f"d{b}")
                        nc.vector.scalar_tensor_tensor(
                            d[b][:], ps3[b][:], -1.0, u[b][:], op0=ALU.mult, op1=ALU.add
                        )
                for b in range(NB):
                    with shift(b):
                        n2[b] = sc_pool.tile([128, 1], f32, tag=f"n2{b}", name=f"n2{b}")
                        dsq[b] = sc_pool.tile([128, FW], f32, tag=f"dsq{b}", name=f"dsq{b}")
                        nc.scalar.activation(dsq[b][:], d[b][:], AF.Square, accum_out=n2[b][:])
                        nmax[b] = sc_pool.tile([128, 1], f32, tag=f"nmax{b}", name=f"nmax{b}")
                        nc.vector.tensor_scalar_max(nmax[b][:], n2[b][:], 1e-24)
                        rr[b] = sc_pool.tile([128, 1], f32, tag=f"rr{b}", name=f"rr{b}")
                        nc.vector.reciprocal_approx_fast(rr[b][:], nmax[b][:])
                        nrm[b] = sc_pool.tile([128, 1], f32, tag=f"nrm{b}", name=f"nrm{b}")
                        nc.scalar.activation(nrm[b][:], rr[b][:], AF.Sqrt, scale=tau * tau)
                        s[b] = sc_pool.tile([128, 1], f32, tag=f"s{b}", name=f"s{b}")
                        nc.scalar.activation(s[b][:], nrm[b][:], AF.Relu, bias=1.0, scale=-1.0)
                for b in range(NB):
                    with shift(b):
                        m_[b] = em_pool.tile([128, FW], f32, tag=f"em{b}", name=f"em{b}")
                        if b == 0:
                            nc.vector.tensor_scalar(
                                m_[b][:], d[b][:], s[b][:], tau, op0=ALU.mult, op1=ALU.add
                            )
                        else:
                            nc.scalar.activation(
                                m_[b][:], d[b][:], AF.Copy, scale=s[b][:], bias=tau
                            )
                for b in range(NB):
                    with shift(b):
                        eb_new[b] = eb_pool.tile([128, FW], f32, tag=f"eb{b}", name=f"eb{b}")
                        nsplit = 2 if b == 0 else 1
                        for hh in range(nsplit):
                            sl = slice(hh * (FW // nsplit), (hh + 1) * (FW // nsplit))
                            nc.vector.ln_bwd_dx(
                                eb_new[b][:, sl], m_[b][:, sl], e[b][:, sl],
                                mean_dyx=0.5, mean_dy=tau / 2.0, scale=2.0,
                            )
                for b in range(NB):
                    with shift(b):
                        e[b], eb[b], q[b], pc[b] = m_[b], eb_new[b], q_new[b], pc_new[b]

            # ---- output: x = Z + e ----
            for b in range(NB):
                xout = sc_pool.tile([128, FW], f32, tag=f"xout{b}")
                nc.vector.affine_then_add(
                    xout[:], e[b][:], Z[b][:], scale=1.0, bias=-tau
                )
                nc.sync.dma_start(d_out.ap()[b * 128 : (b + 1) * 128, :], xout[:])

    nc.finalize()
    return nc


def _get_nc(S: np.ndarray):
    key = hash(S.tobytes())
    if key not in _BUILD_CACHE:
        L = _power_L(S)
        tau = 0.9 / L
        sigma = 0.9 / L
        _BUILD_CACHE[key] = (_build_nc(tau, sigma), tau, sigma)
    return _BUILD_CACHE[key]


def _make_in_maps(X, W1, b1, W2, b2, W3, b3, S, tau, sigma):
    alpha = np.float32(tau * sigma)
    Xflat = np.ascontiguousarray(X.reshape(B_FULL, N_COMBOS)).astype(np.float32)
    # aST packed: alpha * S.T chunks [128, 64] side by side -> [128, 256]
    aST_full = (alpha * S.T).astype(np.float32)  # [512, 64]
    aST = np.concatenate(
        [aST_full[c * 128 : (c + 1) * 128, :] for c in range(NF)], axis=1
    )
    aST = np.ascontiguousarray(aST)
    b1r = np.ascontiguousarray(b1.reshape(8, 128).T).astype(np.float32)
    b2r = np.ascontiguousarray(b2.reshape(8, 128).T).astype(np.float32)
    b3r = np.ascontiguousarray(b3.reshape(4, 128).T).astype(np.float32)
    I128 = np.eye(128, dtype=np.float32)
    shared = {
        "w1": np.ascontiguousarray(W1.astype(np.float32)),
        "b1r": b1r,
        "w2": np.ascontiguousarray(W2.astype(np.float32)),
        "b2r": b2r,
        "w3": np.ascontiguousarray(W3.astype(np.float32)),
        "b3r": b3r,
        "s": np.ascontiguousarray(S.astype(np.float32)),
        "ast": aST,
        "ident": I128,
    }
    in_maps = []
    for c in range(N_CORES):
        xt = np.ascontiguousarray(Xflat[c * BC : (c + 1) * BC, :].T)
        in_maps.append({**shared, "xt": xt})
    return in_maps


def kernel(X, W1, b1, W2, b2, W3, b3, S, batch_size):
    from concourse.bass_utils import run_bass_kernel_spmd

    X = np.asarray(X)
    S = np.asarray(S)
    nc, tau, sigma = _get_nc(np.ascontiguousarray(S.astype(np.float32)))
    in_maps = _make_in_maps(
        X,
        np.asarray(W1),
        np.asarray(b1),
        np.asarray(W2),
        np.asarray(b2),
        np.asarray(W3),
        np.asarray(b3),
        S,
        tau,
        sigma,
    )
    res = run_bass_kernel_spmd(nc, in_maps, core_ids=list(range(N_CORES)))
    out = np.concatenate([res.results[c]["out"] for c in range(N_CORES)], axis=0)
    return out.astype(np.float32)



# revision 3
# speedup vs baseline: 1.1427x; 1.1427x over previous
"""Trainium2 Bass kernel for nn_MatchNet (MLP forward + 60-iter batched PDHG LP solve).

Data-parallel over 8 NeuronCores: batch 2048 -> 256 rows/core. MLP weights and
S are replicated. Each core runs the full unrolled PDHG solve on its shard.

Math (per core, batch rows b=256, n=512 structures, m=64 combos):
    Z = relu(relu(relu(X@W1+b1)@W2+b2)@W3+b3)          (computed in T layout)
    tau = sigma = 0.9/L,  alpha = tau*sigma            (L from host power iter)
    scaled duals p = tau*y1 [m,b]T, q = tau*y2, states e = x-Z, eb = xbar-Z:
      p+  = relu(p + alpha*(S@xbarT - BT))             xbar = Z + eb
      q+  = relu(q - alpha*(Z + eb))
      t1  = p+@S - q+                                  (PSUM)
      d   = e - t1 + tau
      n2  = sum_feat d^2 ; s = relu(1 - tau/max(sqrt(n2),1e-12))
      m_  = s*d ;  e+ = m_ ;  eb+ = 2*m_ - e
    out x = Z + e_final

Layouts: N-folded SBUF tiles [128, 1024]: col b*512+f = (batch 128*bt+p, feat f).
T-folded [128, 1024]: col c*256+j = (feat c*128+p, batch j).
"""

import numpy as np

N_STRUCTS = 512
N_COMBOS = 64
HID = 1024
N_ITERS = 60
N_CORES = 8
B_FULL = 2048
BC = B_FULL // N_CORES  # 256 batch rows per core
NB = BC // 128  # 2 batch sub-tiles
NF = N_STRUCTS // 128  # 4 feature chunks

_BUILD_CACHE = {}


def _power_L(S: np.ndarray) -> float:
    """Replicates reference.py's power iteration for ||K||_2 in float32."""
    S = S.astype(np.float32)
    n = S.shape[1]
    v = np.full((n,), 1.0 / np.sqrt(n), np.float32)
    for _ in range(30):
        v2 = (S.T @ (S @ v) + v).astype(np.float32)
        v = (v2 / np.float32(np.linalg.norm(v2))).astype(np.float32)
    L = np.sqrt(np.vdot(v, (S.T @ (S @ v) + v).astype(np.float32)))
    return float(L)


def _build_nc(tau: float, sigma: float):
    import contextlib

    import concourse.bacc as bacc
    import concourse.mybir as mybir
    import concourse.tile as tile

    f32 = mybir.dt.float32
    f32r = mybir.dt.float32r
    AF = mybir.ActivationFunctionType
    ALU = mybir.AluOpType
    alpha = tau * sigma

    def R(ap):
        return ap.bitcast(f32r)

    nc = bacc.Bacc("TRN2", target_bir_lowering=False, debug=False)

    # ---- DRAM I/O (per-core shapes) ----
    d_XT = nc.dram_tensor("xt", [N_COMBOS, BC], f32, kind="ExternalInput")
    d_W1 = nc.dram_tensor("w1", [N_COMBOS, HID], f32, kind="ExternalInput")
    d_b1 = nc.dram_tensor("b1r", [128, 8], f32, kind="ExternalInput")
    d_W2 = nc.dram_tensor("w2", [HID, HID], f32, kind="ExternalInput")
    d_b2 = nc.dram_tensor("b2r", [128, 8], f32, kind="ExternalInput")
    d_W3 = nc.dram_tensor("w3", [HID, N_STRUCTS], f32, kind="ExternalInput")
    d_b3 = nc.dram_tensor("b3r", [128, 4], f32, kind="ExternalInput")
    d_S = nc.dram_tensor("s", [N_COMBOS, N_STRUCTS], f32, kind="ExternalInput")
    d_aST = nc.dram_tensor("ast", [128, 4 * N_COMBOS], f32, kind="ExternalInput")
    d_I = nc.dram_tensor("ident", [128, 128], f32, kind="ExternalInput")
    d_out = nc.dram_tensor("out", [BC, N_STRUCTS], f32, kind="ExternalOutput")

    FW = N_STRUCTS  # 512 per-b tile width

    with tile.TileContext(nc) as tc:
        stack = contextlib.ExitStack()
        with stack:
            cpool = stack.enter_context(tc.tile_pool(name="consts", bufs=1))

            def cload(dram, shape, tag):
                t = cpool.tile(shape, f32, tag=tag)
                nc.sync.dma_start(t[:], dram.ap())
                return t

            XT = cload(d_XT, [N_COMBOS, BC], "xt")
            W1 = cload(d_W1, [N_COMBOS, HID], "w1")
            b1r = cload(d_b1, [128, 8], "b1r")
            b2r = cload(d_b2, [128, 8], "b2r")
            b3r = cload(d_b3, [128, 4], "b3r")
            S_sb = cload(d_S, [N_COMBOS, N_STRUCTS], "s")
            aST = cload(d_aST, [128, 4 * N_COMBOS], "ast")
            I128 = cload(d_I, [128, 128], "ident")
            # ---- MLP forward, T layout ----
            zt = []  # Z^T tiles [128, BC] x4
            with (
                tc.tile_pool(name="mlp_sb", bufs=1) as mpool,
                tc.tile_pool(name="mlp_ps", bufs=4, space="PSUM") as mpsum,
            ):
                W2 = []
                for k in range(8):
                    t = mpool.tile([128, HID], f32, tag=f"w2_{k}", name=f"w2_{k}")
                    nc.sync.dma_start(t[:], d_W2.ap()[k * 128 : (k + 1) * 128, :])
                    W2.append(t)
                W3 = []
                for k in range(8):
                    t = mpool.tile([128, N_STRUCTS], f32, tag=f"w3_{k}", name=f"w3_{k}")
                    nc.sync.dma_start(t[:], d_W3.ap()[k * 128 : (k + 1) * 128, :])
                    W3.append(t)
                z1t = []
                for t in range(8):
                    ps = mpsum.tile([128, BC], f32, tag="mm")
                    nc.tensor.matmul(
                        ps[:], R(W1[:, t * 128 : (t + 1) * 128]), R(XT[:]), start=True, stop=True
                    )
                    sb = mpool.tile([128, BC], f32, tag=f"z1_{t}")
                    nc.scalar.activation(sb[:], ps[:], AF.Relu, bias=b1r[:, t : t + 1])
                    z1t.append(sb)
                z2t = []
                for t in range(8):
                    ps = mpsum.tile([128, BC], f32, tag="mm")
                    for k in range(8):
                        nc.tensor.matmul(
                            ps[:],
                            R(W2[k][:, t * 128 : (t + 1) * 128]),
                            R(z1t[k][:]),
                            start=(k == 0),
                            stop=(k == 7),
                        )
                    sb = mpool.tile([128, BC], f32, tag=f"z2_{t}")
                    nc.scalar.activation(sb[:], ps[:], AF.Relu, bias=b2r[:, t : t + 1])
                    z2t.append(sb)
                for c in range(NF):
                    ps = mpsum.tile([128, BC], f32, tag="mm")
                    for k in range(8):
                        nc.tensor.matmul(
                            ps[:],
                            R(W3[k][:, c * 128 : (c + 1) * 128]),
                            R(z2t[k][:]),
                            start=(k == 0),
                            stop=(k == 7),
                        )
                    sb = cpool.tile([128, BC], f32, tag=f"zt_{c}")
                    nc.scalar.activation(sb[:], ps[:], AF.Relu, bias=b3r[:, c : c + 1])
                    zt.append(sb)

            # ---- PDHG setup ----
            spool = stack.enter_context(tc.tile_pool(name="setup", bufs=1))
            with tc.tile_pool(name="pd_ps", bufs=1, space="PSUM") as ppool:
                # cSZB = alpha*S@Z^T - alpha*B^T   [64, BC]
                ps = ppool.tile([N_COMBOS, BC], f32, tag="py1")
                for c in range(NF):
                    nc.tensor.matmul(
                        ps[:], R(aST[:, c * 64 : (c + 1) * 64]), R(zt[c][:]),
                        start=(c == 0), stop=False,
                    )
                naI64 = spool.tile([N_COMBOS, N_COMBOS], f32, tag="nai64")
                nc.scalar.activation(naI64[:], I128[:64, :64], AF.Copy, scale=-alpha)
                nc.tensor.matmul(ps[:], R(naI64[:]), R(XT[:]), start=False, stop=True)
                cSZB = spool.tile([N_COMBOS, BC], f32, tag="cszb")
                nc.scalar.activation(cSZB[:], ps[:], AF.Copy)

                # Z per-b in N layout via PE transposes
                Z, naZ = [], []
                for b in range(NB):
                    psz = ppool.tile([128, FW], f32, tag=f"pz{b}")
                    for c in range(NF):
                        nc.tensor.transpose(
                            psz[:, c * 128 : (c + 1) * 128],
                            zt[c][:, b * 128 : (b + 1) * 128],
                            I128[:],
                        )
                    zb = spool.tile([128, FW], f32, tag=f"zn{b}")
                    nc.scalar.activation(zb[:], psz[:], AF.Copy)
                    Z.append(zb)
                    nb_ = spool.tile([128, FW], f32, tag=f"naz{b}")
                    nc.scalar.activation(nb_[:], zb[:], AF.Copy, scale=-alpha)
                    naZ.append(nb_)

            # ---- PDHG state pools ----
            em_pool = stack.enter_context(tc.tile_pool(name="em", bufs=4))
            eb_pool = stack.enter_context(tc.tile_pool(name="eb", bufs=4))
            p_pool = stack.enter_context(tc.tile_pool(name="pp", bufs=4))
            q_pool = stack.enter_context(tc.tile_pool(name="qq", bufs=4))
            sc_pool = stack.enter_context(tc.tile_pool(name="scratch", bufs=4))
            ps_T = stack.enter_context(tc.tile_pool(name="ps_T", bufs=2, space="PSUM"))
            ps_y1 = stack.enter_context(tc.tile_pool(name="ps_y1", bufs=1, space="PSUM"))
            ps_t1 = stack.enter_context(tc.tile_pool(name="ps_t1", bufs=1, space="PSUM"))

            e, eb, pc, q = [], [], [], []
            for b in range(NB):
                t = em_pool.tile([128, FW], f32, tag=f"em{b}")
                nc.scalar.activation(t[:], Z[b][:], AF.Copy, scale=-1.0, bias=tau)
                e.append(t)
                t = eb_pool.tile([128, FW], f32, tag=f"eb{b}")
                nc.scalar.activation(t[:], Z[b][:], AF.Copy, scale=-1.0)
                eb.append(t)
                t = p_pool.tile([N_COMBOS, 128], f32, tag=f"pc{b}")
                nc.vector.tensor_copy(t[:], cSZB[:, b * 128 : (b + 1) * 128])
                pc.append(t)
                t = q_pool.tile([128, FW], f32, tag=f"q{b}")
                nc.gpsimd.memset(t[:], 0.0)
                q.append(t)


            import contextlib as _ctx

            def shift(b):
                # phase-shift chain b0 half an iteration earlier in priority
                return _ctx.nullcontext()

            for it in range(N_ITERS):
                psT, ebT, ps1, p_new, pc_new = [None] * NB, [None] * NB, [None] * NB, [None] * NB, [None] * NB
                w_, h_, q_new, ps3, u = [None] * NB, [None] * NB, [None] * NB, [None] * NB, [None] * NB
                d, n2, dsq, nmax, nrm = [None] * NB, [None] * NB, [None] * NB, [None] * NB, [None] * NB
                rr, rs, s, m_, eb_new = [None] * NB, [None] * NB, [None] * NB, [None] * NB, [None] * NB

                # q+ = relu((q - alpha*Z) - alpha*eb) ; w is off-critical-path
                for b in range(NB):
                    with shift(b):
                        w_[b] = sc_pool.tile([128, FW], f32, tag=f"w{b}", name=f"w{b}")
                        nc.gpsimd.tensor_add(w_[b][:], q[b][:], naZ[b][:])
                first_T = {}
                for b in range(NB):
                    with shift(b):
                        psT[b] = ps_T.tile([128, FW], f32, tag=f"pT{b}", name=f"pT_{b}")
                        for c in range(NF):
                            ti = nc.tensor.transpose(
                                psT[b][:, c * 128 : (c + 1) * 128],
                                eb[b][:, c * 128 : (c + 1) * 128],
                                I128[:],
                            )
                            if c == 0:
                                first_T[b] = ti
                for b in range(NB):
                    with shift(b):
                        ebT[b] = sc_pool.tile([128, FW], f32, tag=f"ebT{b}", name=f"ebT{b}")
                        nc.scalar.activation(ebT[b][:, 0:256], psT[b][:, 0:256], AF.Copy)
                        nc.scalar.activation(ebT[b][:, 256:512], psT[b][:, 256:512], AF.Copy)
                for b in range(NB):
                    with shift(b):
                        h_[b] = sc_pool.tile([128, FW], f32, tag=f"h{b}", name=f"h{b}")
                        nc.vector.scalar_tensor_tensor(
                            h_[b][:], eb[b][:], -alpha, w_[b][:], op0=ALU.mult, op1=ALU.add
                        )
                for b in range(NB):
                    with shift(b):
                        ps1[b] = ps_y1.tile([N_COMBOS, 128], f32, tag=f"py1{b}", name=f"py1_{b}")
                        nc.tensor.matmul(ps1[b][:], R(I128[:64, :64]), R(pc[b][:]), start=True, stop=False)
                        for c in range(NF):
                            nc.tensor.matmul(
                                ps1[b][:],
                                R(aST[:, c * 64 : (c + 1) * 64]),
                                R(ebT[b][:, c * 128 : (c + 1) * 128]),
                                start=False, stop=(c == NF - 1),
                            )
                for b in range(NB):
                    with shift(b):
                        q_new[b] = q_pool.tile([128, FW], f32, tag=f"q{b}", name=f"q{b}")
                        if b == 0:
                            nc.vector.tensor_scalar_max(q_new[b][:], h_[b][:], 0.0)
                        else:
                            nc.scalar.activation(q_new[b][:], h_[b][:], AF.Relu)
                for b in range(NB):
                    with shift(b):
                        p_new[b] = p_pool.tile([N_COMBOS, 128], f32, tag=f"p{b}", name=f"p{b}")
                        nc.scalar.activation(p_new[b][:], ps1[b][:], AF.Relu)
                        pc_new[b] = p_pool.tile([N_COMBOS, 128], f32, tag=f"pc{b}", name=f"pc{b}")
                        nc.gpsimd.tensor_add(
                            pc_new[b][:], p_new[b][:], cSZB[:, b * 128 : (b + 1) * 128]
                        )
                for b in range(NB):
                    with shift(b):
                        u[b] = sc_pool.tile([128, FW], f32, tag=f"u{b}", name=f"u{b}")
                        if b == 0:
                            nc.vector.tensor_add(u[b][:], e[b][:], q_new[b][:])
                        else:
                            nc.gpsimd.tensor_add(u[b][:], e[b][:], q_new[b][:])
                p1_inst = {}
                for b in range(NB):
                    with shift(b):
                        ps3[b] = ps_t1.tile([128, FW], f32, tag=f"pt1{b}", name=f"pt1_{b}")
                        p1_inst[b] = nc.tensor.matmul(
                            ps3[b][:], R(p_new[b][:]), R(S_sb[:]), start=True, stop=True
                        )


                for b in range(NB):
                    with shift(b):
                        d[b] = sc_pool.tile([128, FW], f32, tag=f"d{b}", name=f"d{b}")
                        nc.vector.scalar_tensor_tensor(
                            d[b][:], ps3[b][:], -1.0, u[b][:], op0=ALU.mult, op1=ALU.add
                        )
                for b in range(NB):
                    with shift(b):
                        n2[b] = sc_pool.tile([128, 1], f32, tag=f"n2{b}", name=f"n2{b}")
                        dsq[b] = sc_pool.tile([128, FW], f32, tag=f"dsq{b}", name=f"dsq{b}")
                        nc.scalar.activation(dsq[b][:], d[b][:], AF.Square, accum_out=n2[b][:])
                        nmax[b] = sc_pool.tile([128, 1], f32, tag=f"nmax{b}", name=f"nmax{b}")
                        nc.vector.tensor_scalar_max(nmax[b][:], n2[b][:], 1e-24)
                        rr[b] = sc_pool.tile([128, 1], f32, tag=f"rr{b}", name=f"rr{b}")
                        nc.vector.reciprocal_approx_fast(rr[b][:], nmax[b][:])
                        nrm[b] = sc_pool.tile([128, 1], f32, tag=f"nrm{b}", name=f"nrm{b}")
                        nc.scalar.activation(nrm[b][:], rr[b][:], AF.Sqrt, scale=tau * tau)
                        s[b] = sc_pool.tile([128, 1], f32, tag=f"s{b}", name=f"s{b}")
                        nc.scalar.activation(s[b][:], nrm[b][:], AF.Relu, bias=1.0, scale=-1.0)
                for b in range(NB):
                    with shift(b):
                        m_[b] = em_pool.tile([128, FW], f32, tag=f"em{b}", name=f"em{b}")
                        if b == 0:
                            nc.vector.tensor_scalar(
                                m_[b][:], d[b][:], s[b][:], tau, op0=ALU.mult, op1=ALU.add
                            )
                        else:
                            nc.scalar.activation(
                                m_[b][:], d[b][:], AF.Copy, scale=s[b][:], bias=tau
                            )
                for b in range(NB):
                    with shift(b):
                        eb_new[b] = eb_pool.tile([128, FW], f32, tag=f"eb{b}", name=f"eb{b}")
                        nsplit = 2 if b == 0 else 1
                        for hh in range(nsplit):
                            sl = slice(hh * (FW // nsplit), (hh + 1) * (FW // nsplit))
                            nc.vector.ln_bwd_dx(
                                eb_new[b][:, sl], m_[b][:, sl], e[b][:, sl],
                                mean_dyx=0.5, mean_dy=tau / 2.0, scale=2.0,
                            )
                for b in range(NB):
                    with shift(b):
                        e[b], eb[b], q[b], pc[b] = m_[b], eb_new[b], q_new[b], pc_new[b]

            # ---- output: x = Z + e ----
            for b in range(NB):
                xout = sc_pool.tile([128, FW], f32, tag=f"xout{b}")
                nc.vector.affine_then_add(
                    xout[:], e[b][:], Z[b][:], scale=1.0, bias=-tau
                )
                nc.sync.dma_start(d_out.ap()[b * 128 : (b + 1) * 128, :], xout[:])

    nc.finalize()
    return nc


def _get_nc(S: np.ndarray):
    key = hash(S.tobytes())
    if key not in _BUILD_CACHE:
        L = _power_L(S)
        tau = 0.9 / L
        sigma = 0.9 / L
        _BUILD_CACHE[key] = (_build_nc(tau, sigma), tau, sigma)
    return _BUILD_CACHE[key]


def _make_in_maps(X, W1, b1, W2, b2, W3, b3, S, tau, sigma):
    alpha = np.float32(tau * sigma)
    Xflat = np.ascontiguousarray(X.reshape(B_FULL, N_COMBOS)).astype(np.float32)
    # aST packed: alpha * S.T chunks [128, 64] side by side -> [128, 256]
    aST_full = (alpha * S.T).astype(np.float32)  # [512, 64]
    aST = np.concatenate(
        [aST_full[c * 128 : (c + 1) * 128, :] for c in range(NF)], axis=1
    )
    aST = np.ascontiguousarray(aST)
    b1r = np.ascontiguousarray(b1.reshape(8, 128).T).astype(np.float32)
    b2r = np.ascontiguousarray(b2.reshape(8, 128).T).astype(np.float32)
    b3r = np.ascontiguousarray(b3.reshape(4, 128).T).astype(np.float32)
    I128 = np.eye(128, dtype=np.float32)
    shared = {
        "w1": np.ascontiguousarray(W1.astype(np.float32)),
        "b1r": b1r,
        "w2": np.ascontiguousarray(W2.astype(np.float32)),
        "b2r": b2r,
        "w3": np.ascontiguousarray(W3.astype(np.float32)),
        "b3r": b3r,
        "s": np.ascontiguousarray(S.astype(np.float32)),
        "ast": aST,
        "ident": I128,
    }
    in_maps = []
    for c in range(N_CORES):
        xt = np.ascontiguousarray(Xflat[c * BC : (c + 1) * BC, :].T)
        in_maps.append({**shared, "xt": xt})
    return in_maps


def kernel(X, W1, b1, W2, b2, W3, b3, S, batch_size):
    from concourse.bass_utils import run_bass_kernel_spmd

    X = np.asarray(X)
    S = np.asarray(S)
    nc, tau, sigma = _get_nc(np.ascontiguousarray(S.astype(np.float32)))
    in_maps = _make_in_maps(
        X,
        np.asarray(W1),
        np.asarray(b1),
        np.asarray(W2),
        np.asarray(b2),
        np.asarray(W3),
        np.asarray(b3),
        S,
        tau,
        sigma,
    )
    res = run_bass_kernel_spmd(nc, in_maps, core_ids=list(range(N_CORES)))
    out = np.concatenate([res.results[c]["out"] for c in range(N_CORES)], axis=0)
    return out.astype(np.float32)



# revision 5
# speedup vs baseline: 1.3664x; 1.1957x over previous
"""Trainium2 Bass kernel for nn_MatchNet (MLP forward + 60-iter batched PDHG LP solve).

Data-parallel over 8 NeuronCores: batch 2048 -> 256 rows/core (2 b-tiles of 128).
MLP runs in float32r (1 cyc/row matmuls at N=256). PDHG states are fp16 in
N layout [batch, n] with alpha-scaled prox states so every constant folds into
matmul weights or activation scale/bias.

Math per core (n=512 structures, m=64 combos, tau=sigma=0.9/L, alpha=tau*sigma):
    Z = relu(relu(relu(X@W1+b1)@W2+b2)@W3+b3)
    states: q = tau*y2 (fp16), aeb = alpha*(xbar - Z), E = alpha*(x - Z + tau),
            pc = p + cSZB with p = tau*y1, cSZB = alpha*(S@Z^T - B^T)
    iter:
      w   = q + naZ            (naZ = -alpha*Z)
      h   = w - aeb
      qn  = max(h, 0)
      aebT = transpose(aeb)                       (PE + evac)
      ps1 = I64@pc + sum_c S^T_c @ aebT_c        (PSUM [64,128])
      p   = relu(ps1);  pcn = p + cSZB
      NS3 = p@(a16*S) + (-a16*I)@qn + (-I)@E     (PSUM [128,512] = -alpha*d)
      n2  = sum(NS3^2); rr = 1/max(n2,eps); nr = sqrt(tau^2 alpha^2 rr)
      ns  = min(nr-1, 0)                          (= -s)
      En  = ns*NS3 + alpha*tau                    (Act scale-ptr; = E_new)
      tmp = 2*En - alpha*tau;  aebn = tmp - E
    out x = Z + E/alpha - tau
"""

import numpy as np

N_STRUCTS = 512
N_COMBOS = 64
HID = 1024
N_ITERS = 60
N_CORES = 8
B_FULL = 2048
BC = B_FULL // N_CORES  # 256 batch rows per core
NB = BC // 128  # 2 batch sub-tiles
NF = N_STRUCTS // 128  # 4 feature chunks

# engine assignment knobs
CFG = {
    "w": "pool",      # w = q + naZ
    "h": "dve",       # h = w - aeb
    "qn": "dve",      # qn = max(h,0)
    "tev": "act",     # aebT evacuation: act | dve | split
    "p": "act",       # p = relu(ps1)
    "pc": "dve",      # pcn = p + cSZB
    "n2": {0: "act", 1: "dve"},  # per-b: act Square+accum | dve amr
    "en": "act",      # En from PSUM: act | dve
}

_BUILD_CACHE = {}


def _power_L(S: np.ndarray) -> float:
    """Replicates reference.py's power iteration for ||K||_2 in float32."""
    S = S.astype(np.float32)
    n = S.shape[1]
    v = np.full((n,), 1.0 / np.sqrt(n), np.float32)
    for _ in range(30):
        v2 = (S.T @ (S @ v) + v).astype(np.float32)
        v = (v2 / np.float32(np.linalg.norm(v2))).astype(np.float32)
    L = np.sqrt(np.vdot(v, (S.T @ (S @ v) + v).astype(np.float32)))
    return float(L)


def _build_nc(tau: float, sigma: float):
    import contextlib

    import concourse.bacc as bacc
    import concourse.mybir as mybir
    import concourse.tile as tile

    f32 = mybir.dt.float32
    f32r = mybir.dt.float32r
    f16 = mybir.dt.float16
    AF = mybir.ActivationFunctionType
    ALU = mybir.AluOpType
    alpha = float(np.float32(tau) * np.float32(sigma))
    atau = float(np.float32(alpha) * np.float32(tau))
    t2a2 = float((np.float32(tau) * np.float32(alpha)) ** 2)

    nc = bacc.Bacc("TRN2", target_bir_lowering=False, debug=False)

    def creg(v):
        key = (f32, v)
        if key not in nc.const_aps.aps:
            t = nc.alloc_sbuf_tensor(f"constx-{v}", [128, 1], f32)
            nc.gpsimd.memset(t.ap(), v)
            nc.const_aps.aps[key] = t.ap()
        return v

    creg(atau)
    creg(-tau)

    # ---- DRAM I/O (per-core shapes) ----
    d_XT = nc.dram_tensor("xt", [N_COMBOS, BC], f32r, kind="ExternalInput")
    d_W1 = nc.dram_tensor("w1", [N_COMBOS, HID], f32r, kind="ExternalInput")
    d_b1 = nc.dram_tensor("b1r", [128, 8], f32, kind="ExternalInput")
    d_W2 = nc.dram_tensor("w2", [HID, HID], f32r, kind="ExternalInput")
    d_b2 = nc.dram_tensor("b2r", [128, 8], f32, kind="ExternalInput")
    d_W3 = nc.dram_tensor("w3", [HID, N_STRUCTS], f32r, kind="ExternalInput")
    d_b3 = nc.dram_tensor("b3r", [128, 4], f32, kind="ExternalInput")
    d_aST = nc.dram_tensor("ast", [128, NF * N_COMBOS], f32r, kind="ExternalInput")
    d_ST16 = nc.dram_tensor("st16", [128, NF * N_COMBOS], f16, kind="ExternalInput")
    d_AS16 = nc.dram_tensor("as16", [N_COMBOS, N_STRUCTS], f16, kind="ExternalInput")
    d_nAI16 = nc.dram_tensor("nai16", [128, 128], f16, kind="ExternalInput")
    d_nI16 = nc.dram_tensor("ni16", [128, 128], f16, kind="ExternalInput")
    d_I64 = nc.dram_tensor("i64_16", [N_COMBOS, N_COMBOS], f16, kind="ExternalInput")
    d_I16 = nc.dram_tensor("i16", [128, 128], f16, kind="ExternalInput")
    d_Ir = nc.dram_tensor("identr", [128, 128], f32r, kind="ExternalInput")
    d_out = nc.dram_tensor("out", [BC, N_STRUCTS], f32, kind="ExternalOutput")

    FW = N_STRUCTS  # 512 per-b tile width

    with tile.TileContext(nc) as tc:
        stack = contextlib.ExitStack()
        with stack:
            cpool = stack.enter_context(tc.tile_pool(name="consts", bufs=1))

            def cload(dram, shape, tag, dt):
                t = cpool.tile(shape, dt, tag=tag)
                nc.sync.dma_start(t[:], dram.ap())
                return t

            XT = cload(d_XT, [N_COMBOS, BC], "xt", f32r)
            W1 = cload(d_W1, [N_COMBOS, HID], "w1", f32r)
            b1r = cload(d_b1, [128, 8], "b1r", f32)
            b2r = cload(d_b2, [128, 8], "b2r", f32)
            b3r = cload(d_b3, [128, 4], "b3r", f32)
            aST = cload(d_aST, [128, NF * N_COMBOS], "ast", f32r)
            ST16 = cload(d_ST16, [128, NF * N_COMBOS], "st16", f16)
            AS16 = cload(d_AS16, [N_COMBOS, N_STRUCTS], "as16", f16)
            nAI16 = cload(d_nAI16, [128, 128], "nai16", f16)
            nI16 = cload(d_nI16, [128, 128], "ni16", f16)
            I64 = cload(d_I64, [N_COMBOS, N_COMBOS], "i64_16", f16)
            I16 = cload(d_I16, [128, 128], "i16", f16)
            Ir = cload(d_Ir, [128, 128], "identr", f32r)

            # ---- MLP forward (float32r, T layout) ----
            zt = []  # Z^T tiles [128, BC] x4, f32r
            with (
                tc.tile_pool(name="mlp_sb", bufs=1) as mpool,
                tc.tile_pool(name="mlp_ps", bufs=4, space="PSUM") as mpsum,
            ):
                W2 = []
                for k in range(8):
                    t = mpool.tile([128, HID], f32r, tag=f"w2_{k}", name=f"w2_{k}")
                    nc.sync.dma_start(t[:], d_W2.ap()[k * 128 : (k + 1) * 128, :])
                    W2.append(t)
                W3 = []
                for k in range(8):
                    t = mpool.tile([128, N_STRUCTS], f32r, tag=f"w3_{k}", name=f"w3_{k}")
                    nc.sync.dma_start(t[:], d_W3.ap()[k * 128 : (k + 1) * 128, :])
                    W3.append(t)
                z1t = []
                for t in range(8):
                    ps = mpsum.tile([128, BC], f32, tag="mm")
                    nc.tensor.matmul(
                        ps[:], W1[:, t * 128 : (t + 1) * 128], XT[:], start=True, stop=True
                    )
                    sb = mpool.tile([128, BC], f32r, tag=f"z1_{t}")
                    nc.scalar.activation(sb[:], ps[:], AF.Relu, bias=b1r[:, t : t + 1])
                    z1t.append(sb)
                z2t = []
                for t in range(8):
                    ps = mpsum.tile([128, BC], f32, tag="mm")
                    for k in range(8):
                        nc.tensor.matmul(
                            ps[:],
                            W2[k][:, t * 128 : (t + 1) * 128],
                            z1t[k][:],
                            start=(k == 0),
                            stop=(k == 7),
                        )
                    sb = mpool.tile([128, BC], f32r, tag=f"z2_{t}")
                    nc.scalar.activation(sb[:], ps[:], AF.Relu, bias=b2r[:, t : t + 1])
                    z2t.append(sb)
                for c in range(NF):
                    ps = mpsum.tile([128, BC], f32, tag="mm")
                    for k in range(8):
                        nc.tensor.matmul(
                            ps[:],
                            W3[k][:, c * 128 : (c + 1) * 128],
                            z2t[k][:],
                            start=(k == 0),
                            stop=(k == 7),
                        )
                    sb = cpool.tile([128, BC], f32r, tag=f"zt_{c}")
                    nc.scalar.activation(sb[:], ps[:], AF.Relu, bias=b3r[:, c : c + 1])
                    zt.append(sb)

            # ---- PDHG setup ----
            spool = stack.enter_context(tc.tile_pool(name="setup", bufs=1))
            with tc.tile_pool(name="pd_ps", bufs=1, space="PSUM") as ppool:
                # cSZB16 = alpha*(S@Z^T - B^T)   [64, BC] fp16
                ps = ppool.tile([N_COMBOS, BC], f32, tag="py1")
                for c in range(NF):
                    nc.tensor.matmul(
                        ps[:], aST[:, c * 64 : (c + 1) * 64], zt[c][:],
                        start=(c == 0), stop=False,
                    )
                naI64 = spool.tile([N_COMBOS, N_COMBOS], f32r, tag="nai64")
                nc.scalar.activation(naI64[:], Ir[:64, :64].bitcast(f32), AF.Copy, scale=-alpha)
                nc.tensor.matmul(ps[:], naI64[:], XT[:], start=False, stop=True)
                cSZB = spool.tile([N_COMBOS, BC], f16, tag="cszb")
                nc.scalar.activation(cSZB[:], ps[:], AF.Copy)

                # Z per-b in N layout (f32) via PE transposes
                Zf = []
                for b in range(NB):
                    psz = ppool.tile([128, FW], f32r, tag=f"pz{b}")
                    for c in range(NF):
                        nc.tensor.transpose(
                            psz[:, c * 128 : (c + 1) * 128],
                            zt[c][:, b * 128 : (b + 1) * 128],
                            Ir[:],
                        )
                    zb = spool.tile([128, FW], f32, tag=f"zn{b}")
                    nc.scalar.activation(zb[:], psz[:].bitcast(f32), AF.Copy)
                    Zf.append(zb)

            # constants + state init (fp16)
            naZ = []
            for b in range(NB):
                t = spool.tile([128, FW], f16, tag=f"naz{b}")
                nc.vector.tensor_scalar(t[:], Zf[b][:], -alpha, 0.0, op0=ALU.mult, op1=ALU.add)
                naZ.append(t)

            # ---- PDHG state pools ----
            e_pool = stack.enter_context(tc.tile_pool(name="ep", bufs=3))
            aeb_pool = stack.enter_context(tc.tile_pool(name="aebp", bufs=3))
            q_pool = stack.enter_context(tc.tile_pool(name="qp", bufs=3))
            p_pool = stack.enter_context(tc.tile_pool(name="pp", bufs=3))
            sc_pool = stack.enter_context(tc.tile_pool(name="scratch", bufs=3))
            ps_T = stack.enter_context(tc.tile_pool(name="ps_T", bufs=1, space="PSUM"))
            ps_y1 = stack.enter_context(tc.tile_pool(name="ps_y1", bufs=1, space="PSUM"))
            ps_3 = stack.enter_context(tc.tile_pool(name="ps_3", bufs=1, space="PSUM"))

            E, aeb, q, pc = [], [], [], []
            for b in range(NB):
                t = e_pool.tile([128, FW], f16, tag=f"e{b}")
                nc.vector.tensor_scalar(t[:], Zf[b][:], -alpha, atau, op0=ALU.mult, op1=ALU.add)
                E.append(t)
                t = aeb_pool.tile([128, FW], f16, tag=f"aeb{b}")
                nc.vector.tensor_scalar(t[:], Zf[b][:], -alpha, 0.0, op0=ALU.mult, op1=ALU.add)
                aeb.append(t)
                t = q_pool.tile([128, FW], f16, tag=f"q{b}")
                nc.gpsimd.memset(t[:], 0.0)
                q.append(t)
                t = p_pool.tile([N_COMBOS, 128], f16, tag=f"pc{b}")
                nc.vector.tensor_copy(t[:], cSZB[:, b * 128 : (b + 1) * 128])
                pc.append(t)

            def tt(engine, out, a, bb, op):
                if engine == "pool":
                    nc.gpsimd.tensor_tensor(out, a, bb, op)
                else:
                    nc.vector.tensor_tensor(out, a, bb, op)

            for it in range(N_ITERS):
                w_ = [None] * NB
                h_ = [None] * NB
                qn = [None] * NB
                psT = [None] * NB
                aebT = [None] * NB
                ps1 = [None] * NB
                p_ = [None] * NB
                pcn = [None] * NB
                ns3 = [None] * NB
                n2 = [None] * NB
                n2c = [None] * NB
                rr = [None] * NB
                nr = [None] * NB
                ns = [None] * NB
                En = [None] * NB
                tmp = [None] * NB
                aebn = [None] * NB

                # q chain
                for b in range(NB):
                    w_[b] = sc_pool.tile([128, FW], f16, tag=f"w{b}", name=f"w{b}")
                    tt(CFG["w"], w_[b][:], q[b][:], naZ[b][:], ALU.add)
                for b in range(NB):
                    h_[b] = sc_pool.tile([128, FW], f16, tag=f"h{b}", name=f"h{b}")
                    tt(CFG["h"], h_[b][:], w_[b][:], aeb[b][:], ALU.subtract)
                for b in range(NB):
                    qn[b] = q_pool.tile([128, FW], f16, tag=f"q{b}", name=f"qn{b}")
                    if CFG["qn"] == "act":
                        nc.scalar.activation(qn[b][:], h_[b][:], AF.Relu)
                    elif CFG["qn"] == "pool":
                        nc.gpsimd.tensor_scalar(qn[b][:], h_[b][:], 0.0, None, op0=ALU.max)
                    else:
                        nc.vector.tensor_scalar_max(qn[b][:], h_[b][:], 0.0)

                # transpose aeb -> aebT
                for b in range(NB):
                    psT[b] = ps_T.tile([128, FW], f16, tag=f"pT{b}", name=f"pT{b}")
                    for c in range(NF):
                        nc.tensor.transpose(
                            psT[b][:, c * 128 : (c + 1) * 128],
                            aeb[b][:, c * 128 : (c + 1) * 128],
                            I16[:],
                        )
                for b in range(NB):
                    aebT[b] = sc_pool.tile([128, FW], f16, tag=f"aebT{b}", name=f"aebT{b}")
                    if CFG["tev"] == "act":
                        nc.scalar.activation(aebT[b][:], psT[b][:], AF.Copy)
                    elif CFG["tev"] == "dve":
                        nc.vector.tensor_copy(aebT[b][:], psT[b][:])
                    else:  # split
                        nc.scalar.activation(aebT[b][:, 0:256], psT[b][:, 0:256], AF.Copy)
                        nc.vector.tensor_copy(aebT[b][:, 256:512], psT[b][:, 256:512])

                # dual p update
                for b in range(NB):
                    ps1[b] = ps_y1.tile([N_COMBOS, 128], f32, tag=f"py{b}", name=f"py{b}")
                    nc.tensor.matmul(ps1[b][:], I64[:], pc[b][:], start=True, stop=False)
                    for c in range(NF):
                        nc.tensor.matmul(
                            ps1[b][:],
                            ST16[:, c * 64 : (c + 1) * 64],
                            aebT[b][:, c * 128 : (c + 1) * 128],
                            start=False, stop=(c == NF - 1),
                        )
                for b in range(NB):
                    p_[b] = p_pool.tile([N_COMBOS, 128], f16, tag=f"p{b}", name=f"p{b}")
                    if CFG["p"] == "act":
                        nc.scalar.activation(p_[b][:], ps1[b][:], AF.Relu)
                    else:
                        nc.vector.tensor_scalar_max(p_[b][:], ps1[b][:], 0.0)
                for b in range(NB):
                    pcn[b] = p_pool.tile([N_COMBOS, 128], f16, tag=f"pc{b}", name=f"pc{b}")
                    if CFG["pc"] == "pool":
                        nc.gpsimd.tensor_tensor(
                            pcn[b][:], p_[b][:], cSZB[:, b * 128 : (b + 1) * 128], ALU.add
                        )
                    else:
                        nc.vector.tensor_tensor(
                            pcn[b][:], p_[b][:], cSZB[:, b * 128 : (b + 1) * 128], ALU.add
                        )

                # NS3 = p@(a16 S) + (-a16 I)@qn + (-I)@E   (= -alpha*d)
                for b in range(NB):
                    ns3[b] = ps_3.tile([128, FW], f32, tag=f"p3{b}", name=f"p3{b}")
                    nc.tensor.matmul(ns3[b][:], p_[b][:], AS16[:], start=True, stop=False)
                    nc.tensor.matmul(ns3[b][:], nAI16[:], qn[b][:], start=False, stop=False)
                    nc.tensor.matmul(ns3[b][:], nI16[:], E[b][:], start=False, stop=True)

                # norm: n2 = sum(NS3^2)
                for b in range(NB):
                    n2[b] = sc_pool.tile([128, 1], f32, tag=f"n2{b}", name=f"n2{b}")
                    eng = CFG["n2"][b] if isinstance(CFG["n2"], dict) else CFG["n2"]
                    dsq = sc_pool.tile([128, FW], f32, tag=f"dsq{b}", name=f"dsq{b}")
                    if eng == "act":
                        nc.scalar.activation(dsq[:], ns3[b][:], AF.Square, accum_out=n2[b][:])
                    else:
                        nc.vector.affine_mul_reduce(
                            dsq[:], n2[b][:], ns3[b][:], ns3[b][:], scale=1.0, bias=0.0
                        )
                for b in range(NB):
                    n2c[b] = sc_pool.tile([128, 1], f32, tag=f"n2c{b}", name=f"n2c{b}")
                    nc.vector.tensor_scalar_max(n2c[b][:], n2[b][:], 1e-12)
                    rr[b] = sc_pool.tile([128, 1], f32, tag=f"rr{b}", name=f"rr{b}")
                    nc.vector.reciprocal_approx_fast(rr[b][:], n2c[b][:])
                    nr[b] = sc_pool.tile([128, 1], f32, tag=f"nr{b}", name=f"nr{b}")
                    nc.scalar.activation(nr[b][:], rr[b][:], AF.Sqrt, scale=t2a2)
                    ns[b] = sc_pool.tile([128, 1], f32, tag=f"ns{b}", name=f"ns{b}")
                    nc.vector.tensor_scalar(
                        ns[b][:], nr[b][:], 1.0, 0.0, op0=ALU.subtract, op1=ALU.min
                    )

                # En = ns*NS3 + alpha*tau  (fp16, = E_new)
                for b in range(NB):
                    En[b] = e_pool.tile([128, FW], f16, tag=f"e{b}", name=f"en{b}")
                    if CFG["en"] == "act":
                        nc.scalar.activation(
                            En[b][:], ns3[b][:], AF.Copy, scale=ns[b][:], bias=atau
                        )
                    else:
                        nc.vector.tensor_scalar(
                            En[b][:], ns3[b][:], ns[b][:], atau, op0=ALU.mult, op1=ALU.add
                        )

                # aebn = 2*En - alpha*tau - E
                for b in range(NB):
                    tmp[b] = sc_pool.tile([128, FW], f16, tag=f"tmp{b}", name=f"tmp{b}")
                    nc.vector.tensor_scalar(
                        tmp[b][:], En[b][:], 2.0, -atau, op0=ALU.mult, op1=ALU.add
                    )
                for b in range(NB):
                    aebn[b] = aeb_pool.tile([128, FW], f16, tag=f"aeb{b}", name=f"aebn{b}")
                    nc.vector.tensor_tensor(aebn[b][:], tmp[b][:], E[b][:], ALU.subtract)

                for b in range(NB):
                    E[b], aeb[b], q[b], pc[b] = En[b], aebn[b], qn[b], pcn[b]

            # ---- output: x = Z + E/alpha - tau ----
            for b in range(NB):
                xe = sc_pool.tile([128, FW], f32, tag=f"xe{b}")
                nc.scalar.activation(xe[:], E[b][:], AF.Copy, scale=1.0 / alpha, bias=-tau)
                xout = sc_pool.tile([128, FW], f32, tag=f"xo{b}")
                nc.vector.tensor_tensor(xout[:], xe[:], Zf[b][:], ALU.add)
                nc.sync.dma_start(d_out.ap()[b * 128 : (b + 1) * 128, :], xout[:])

    nc.finalize()
    return nc


def _get_nc(S: np.ndarray):
    key = hash(S.tobytes())
    if key not in _BUILD_CACHE:
        L = _power_L(S)
        tau = 0.9 / L
        sigma = 0.9 / L
        _BUILD_CACHE[key] = (_build_nc(tau, sigma), tau, sigma)
    return _BUILD_CACHE[key]


def _make_in_maps(X, W1, b1, W2, b2, W3, b3, S, tau, sigma):
    f32 = np.float32
    alpha = np.float32(tau) * np.float32(sigma)
    a16 = np.float16(alpha).astype(f32)
    Xflat = np.ascontiguousarray(X.reshape(B_FULL, N_COMBOS)).astype(f32)
    S = S.astype(f32)
    # aST packed: alpha * S.T chunks [128, 64] side by side -> [128, 256]
    aST_full = (alpha * S.T).astype(f32)  # [512, 64]
    aST = np.ascontiguousarray(
        np.concatenate([aST_full[c * 128 : (c + 1) * 128, :] for c in range(NF)], axis=1)
    )
    ST16_full = S.T.astype(np.float16)  # exact 0/1
    ST16 = np.ascontiguousarray(
        np.concatenate([ST16_full[c * 128 : (c + 1) * 128, :] for c in range(NF)], axis=1)
    )
    AS16 = np.ascontiguousarray((a16 * S).astype(np.float16))
    I128 = np.eye(128, dtype=f32)
    shared = {
        "w1": np.ascontiguousarray(W1.astype(f32)),
        "b1r": np.ascontiguousarray(b1.reshape(8, 128).T).astype(f32),
        "w2": np.ascontiguousarray(W2.astype(f32)),
        "b2r": np.ascontiguousarray(b2.reshape(8, 128).T).astype(f32),
        "w3": np.ascontiguousarray(W3.astype(f32)),
        "b3r": np.ascontiguousarray(b3.reshape(4, 128).T).astype(f32),
        "ast": aST,
        "st16": ST16,
        "as16": AS16,
        "nai16": np.ascontiguousarray((-a16 * I128).astype(np.float16)),
        "ni16": np.ascontiguousarray((-I128).astype(np.float16)),
        "i64_16": np.eye(N_COMBOS, dtype=np.float16),
        "i16": I128.astype(np.float16),
        "identr": I128,
    }
    in_maps = []
    for c in range(N_CORES):
        xt = np.ascontiguousarray(Xflat[c * BC : (c + 1) * BC, :].T)
        in_maps.append({**shared, "xt": xt})
    return in_maps


def kernel(X, W1, b1, W2, b2, W3, b3, S, batch_size):
    from concourse.bass_utils import run_bass_kernel_spmd

    X = np.asarray(X)
    S = np.asarray(S)
    nc, tau, sigma = _get_nc(np.ascontiguousarray(S.astype(np.float32)))
    in_maps = _make_in_maps(
        X,
        np.asarray(W1),
        np.asarray(b1),
        np.asarray(W2),
        np.asarray(b2),
        np.asarray(W3),
        np.asarray(b3),
        S,
        tau,
        sigma,
    )
    res = run_bass_kernel_spmd(nc, in_maps, core_ids=list(range(N_CORES)))
    out = np.concatenate([res.results[c]["out"] for c in range(N_CORES)], axis=0)
    return out.astype(np.float32)
